# revision 5
# baseline (speedup 1.0000x reference)
"""Trainium2 Bass kernel for the ContinuousVariableQNN problem (v2).

Math reduction (validated against the jax reference on host):
  The reference builds a 256x256 symplectic matrix S from params, then
    mu   = mu0 @ S.T   with mu0[:, 0::2] = 2*inputs (odd cols zero)
    n    = (dsum + mu_x^2 + mu_p^2) / (2*hbar) - 0.5
  Because mu0's p-quadrature entries are all zero, the big matmul collapses to
    mu_dev = inputs @ Ms          with Ms[i, j] = S[j, 2*i]   ([128, 256])
  (factor 2 from displacement and the 1/4 normalization cancel), and
    n[b, m] = mu_dev[b, 2m]^2 + mu_dev[b, 2m+1]^2 + bias[m]
  with bias[m] = (diag(S S^T)[2m] + diag(S S^T)[2m+1])/4 - 0.5 (a constant).

v2 device strategy (transposed orientation, fp16 operands, bf16 output):
  Host pre-transposes X so tiles arrive as X^T [feature, batch]: no PE
  transposes at all.  Per 512-batch-column chunk, two stationary-weight
  matmuls (Mx = Ms[:, 0::2], Mp = Ms[:, 1::2], both fp16):
      mux^T = Mx^T @ X^T-chunk   -> PSUM bank   [mode, batch]
      mup^T = Mp^T @ X^T-chunk   -> PSUM bank
  Mode index lands on partitions, so bias is a per-partition scalar and no
  stride-2 de-interleave is needed.  Tail per chunk pair:
      ACT   : sqx = Square(mux^T)              (PSUM f32 -> SBUF bf16)
      DVE   : sqp = mup^T * mup^T              (PSUM f32 -> SBUF bf16)
      DVE/GPSIMD (alternating): out = (sqx + bias) + sqp   (one
              scalar_tensor_tensor, all-bf16 SBUF -> DVE 4x mode)
  IO is fp16 in / bf16 out, halving HBM traffic vs f32 (DMA floor ~23us/core).
  Host-simulated pipeline rel err vs f64 reference: 8.0e-3 (gate 2e-2).
  Input pieces ride the SP HWDGE queue, output pieces the ACT HWDGE queue,
  4KB contiguous per partition per piece.
"""

import ml_dtypes
import numpy as np

import concourse.bass as bass
import concourse.mybir as mybir
import concourse.tile as tile
from concourse import bacc
from concourse.bass_utils import run_bass_kernel_spmd

N_QUMODES = 128
N_LAYERS = 8
BATCH = 131072
N_CORES = 8
ROWS = BATCH // N_CORES          # 16384 batch columns per core (free dim)
PIECES = 8                       # DMA granularity: 2048 cols = 4KB/partition
PC = ROWS // PIECES              # 2048
CHUNK = 512                      # matmul free dim (one PSUM bank of f32)
PAIRS = ROWS // (2 * CHUNK)      # 16 chunk pairs
F32 = mybir.dt.float32
F16 = mybir.dt.float16
BF16 = mybir.dt.bfloat16


def host_prep(params: np.ndarray):
    """Build Mxp [128, 256] fp16 (Mx | Mp) and bias [128, 1] f32 on host."""
    L, N = N_LAYERS, N_QUMODES
    p = params.reshape(L, N, 3).astype(np.float64)
    th1, r, th2 = p[..., 0], p[..., 1], p[..., 2]

    def rot(th):
        c, s = np.cos(th), np.sin(th)
        return np.stack([np.stack([c, -s], -1), np.stack([s, c], -1)], -2)

    z = np.zeros_like(r)
    sq = np.stack([np.stack([np.exp(-r), z], -1),
                   np.stack([z, np.exp(r)], -1)], -2)
    blk = np.einsum('lnab,lnbc,lncd->lnad', rot(th2), sq, rot(th1))

    t = np.cos(np.pi / 4)
    rr = np.sin(np.pi / 4)
    BS4 = np.array([[t, 0., -rr, 0.],
                    [0., t, 0., -rr],
                    [rr, 0., t, 0.],
                    [0., rr, 0., t]])
    C = np.eye(2 * N)
    for i in range(N - 1):
        C[2 * i:2 * i + 4, :] = BS4 @ C[2 * i:2 * i + 4, :]

    S = np.eye(2 * N)
    idx = np.arange(N)
    for l in range(L):
        D = np.zeros((N, 2, N, 2))
        D[idx, :, idx, :] = blk[l]
        S = C @ (D.reshape(2 * N, 2 * N) @ S)

    # mu_dev[b, j] = (inputs @ Ms)[b, j] with Ms = S[:, 0::2].T  [128, 256].
    Ms = S[:, 0::2].T
    Mx = Ms[:, 0::2]                 # [128 feat, 128 mode] x-quadrature
    Mp = Ms[:, 1::2]                 # p-quadrature
    mxp = np.ascontiguousarray(
        np.concatenate([Mx, Mp], axis=1)).astype(np.float16)   # [128, 256]

    dV = (S ** 2).sum(axis=1)                                  # [256]
    bias = ((dV[0::2] + dV[1::2]) / 4.0 - 0.5)
    bias_col = np.ascontiguousarray(bias.reshape(128, 1)).astype(np.float32)
    return mxp, bias_col


def build_bass():
    nc = bacc.Bacc("TRN2", target_bir_lowering=False, debug=False,
                   num_devices=N_CORES)

    x_d = nc.dram_tensor("x", [128, ROWS], F16, kind="ExternalInput")
    mxp_d = nc.dram_tensor("mxp", [128, 256], F16, kind="ExternalInput")
    bias_d = nc.dram_tensor("bias", [128, 1], F32, kind="ExternalInput")
    out_d = nc.dram_tensor("out", [128, ROWS], BF16, kind="ExternalOutput")

    x_v = x_d.ap().rearrange("p (k c) -> k p c", c=PC)     # [8][128, 2048]
    out_v = out_d.ap().rearrange("p (k c) -> k p c", c=PC)

    with tile.TileContext(nc) as tc:
        with (
            tc.tile_pool(name="const", bufs=1) as const_pool,
            tc.tile_pool(name="xin", bufs=PIECES) as xin_pool,
            tc.tile_pool(name="oout", bufs=4) as oout_pool,
            tc.tile_pool(name="sq", bufs=4) as sq_pool,
            tc.tile_pool(name="mup", bufs=2, space="PSUM") as mup_pool,
        ):
            mxp_sb = const_pool.tile([128, 256], F16)
            nc.sync.dma_start(out=mxp_sb, in_=mxp_d.ap())
            bias_sb = const_pool.tile([128, 1], F32)
            nc.sync.dma_start(out=bias_sb, in_=bias_d.ap())

            x_tiles = []
            for k in range(PIECES):
                x_sb = xin_pool.tile([128, PC], F16, tag="x_sb",
                                     name=f"x_sb_{k}")
                if k == 0:
                    # halve the first transfer so the PE can start sooner
                    nc.sync.dma_start(out=x_sb[:, 0:PC // 2],
                                      in_=x_v[k][:, 0:PC // 2])
                    nc.sync.dma_start(out=x_sb[:, PC // 2:],
                                      in_=x_v[k][:, PC // 2:])
                else:
                    nc.sync.dma_start(out=x_sb, in_=x_v[k])
                x_tiles.append(x_sb)

            out_tiles = {}
            for g in range(PAIRS):
                k, gk = divmod(g, PAIRS // PIECES)    # piece idx, pair in piece
                if gk == 0:
                    out_tiles[k] = oout_pool.tile([128, 4, CHUNK], BF16,
                                                  tag="o_sb", name=f"o_sb_{k}")
                x_sb = x_tiles[k]
                c0 = 2 * gk * CHUNK                   # col offset in piece
                c1 = c0 + CHUNK

                # 4 banks: [mux0, mup0, mux1, mup1]; same-weight matmuls
                # adjacent so the PE reloads weights every 2 instead of 1.
                ps = mup_pool.tile([128, 4, CHUNK], F32)
                nc.tensor.matmul(ps[:, 0, :], mxp_sb[:, 0:128],
                                 x_sb[:, c0:c0 + CHUNK], start=True, stop=True)
                nc.tensor.matmul(ps[:, 2, :], mxp_sb[:, 0:128],
                                 x_sb[:, c1:c1 + CHUNK], start=True, stop=True)
                nc.tensor.matmul(ps[:, 1, :], mxp_sb[:, 128:256],
                                 x_sb[:, c0:c0 + CHUNK], start=True, stop=True)
                nc.tensor.matmul(ps[:, 3, :], mxp_sb[:, 128:256],
                                 x_sb[:, c1:c1 + CHUNK], start=True, stop=True)

                # ACT squares banks 0-2 (mux0, mup0, mux1) in one pass; an
                # engine instruction may read only ONE input from PSUM, so
                # the 4th bank goes DVE copy -> SBUF bf16 -> self-multiply.
                sq3 = sq_pool.tile([128, 3, CHUNK], BF16, tag="sq3",
                                   name=f"sq3_{g}")
                nc.scalar.activation(sq3, ps[:, 0:3, :],
                                     mybir.ActivationFunctionType.Square)
                mup1 = sq_pool.tile([128, CHUNK], BF16, tag="mup1",
                                    name=f"mup1_{g}")
                nc.vector.tensor_copy(mup1, ps[:, 3, :])
                # gpsimd squares the copied bank (scalar_tensor_tensor is
                # not available on Pool, but plain tensor_tensor is).
                sqp1 = sq_pool.tile([128, CHUNK], BF16, tag="sqp1",
                                    name=f"sqp1_{g}")
                nc.gpsimd.tensor_tensor(out=sqp1, in0=mup1, in1=mup1,
                                        op=mybir.AluOpType.mult)

                # Combine: out = (sqx + bias) + sqp, bias per-partition
                # scalar; all-bf16 SBUF operands keep DVE in fast mode.
                o_sb = out_tiles[k]
                nc.vector.scalar_tensor_tensor(
                    out=o_sb[:, 2 * gk, :], in0=sq3[:, 0, :], scalar=bias_sb,
                    in1=sq3[:, 1, :], op0=mybir.AluOpType.add,
                    op1=mybir.AluOpType.add)
                nc.vector.scalar_tensor_tensor(
                    out=o_sb[:, 2 * gk + 1, :], in0=sq3[:, 2, :],
                    scalar=bias_sb, in1=sqp1,
                    op0=mybir.AluOpType.add, op1=mybir.AluOpType.add)

                if gk == PAIRS // PIECES - 1:
                    # Outputs ride the SP queue too: in+out serialize there
                    # at exactly the aggregate DMA floor, and the ACT
                    # sequencer keeps all its time for the squares.
                    nc.sync.dma_start(out=out_v[k],
                                      in_=out_tiles.pop(k).rearrange(
                                          "p a b -> p (a b)"))

    nc.compile()
    return nc


_NC_CACHE = None


def make_in_maps(X: np.ndarray, params: np.ndarray):
    mxp, bias_col = host_prep(params)
    xt = np.ascontiguousarray(X.astype(np.float16).T)     # [128, BATCH]
    return [
        {"x": np.ascontiguousarray(xt[:, i * ROWS:(i + 1) * ROWS]),
         "mxp": mxp, "bias": bias_col}
        for i in range(N_CORES)
    ]


def assemble_output(results) -> np.ndarray:
    full = np.concatenate([r["out"] for r in results], axis=1)  # [128, BATCH]
    return np.ascontiguousarray(full.T.astype(np.float32))


def kernel(**inputs: np.ndarray) -> np.ndarray:
    global _NC_CACHE
    X = np.asarray(inputs["inputs"], dtype=np.float32)
    params = np.asarray(inputs["params"], dtype=np.float32)
    assert X.shape == (BATCH, N_QUMODES)

    if _NC_CACHE is None:
        _NC_CACHE = build_bass()
    nc = _NC_CACHE

    in_maps = make_in_maps(X, params)
    res = run_bass_kernel_spmd(nc, in_maps, core_ids=list(range(N_CORES)))
    return assemble_output(res.results)


# revision 8
# speedup vs baseline: 1.3620x; 1.3620x over previous
"""Trainium2 Bass kernel for the ContinuousVariableQNN problem (v2).

Math reduction (validated against the jax reference on host):
  The reference builds a 256x256 symplectic matrix S from params, then
    mu   = mu0 @ S.T   with mu0[:, 0::2] = 2*inputs (odd cols zero)
    n    = (dsum + mu_x^2 + mu_p^2) / (2*hbar) - 0.5
  Because mu0's p-quadrature entries are all zero, the big matmul collapses to
    mu_dev = inputs @ Ms          with Ms[i, j] = S[j, 2*i]   ([128, 256])
  (factor 2 from displacement and the 1/4 normalization cancel), and
    n[b, m] = mu_dev[b, 2m]^2 + mu_dev[b, 2m+1]^2 + bias[m]
  with bias[m] = (diag(S S^T)[2m] + diag(S S^T)[2m+1])/4 - 0.5 (a constant).

v2 device strategy (transposed orientation, fp16 operands, bf16 output):
  Host pre-transposes X so tiles arrive as X^T [feature, batch]: no PE
  transposes at all.  Per 512-batch-column chunk, two stationary-weight
  matmuls (Mx = Ms[:, 0::2], Mp = Ms[:, 1::2], both fp16):
      mux^T = Mx^T @ X^T-chunk   -> PSUM bank   [mode, batch]
      mup^T = Mp^T @ X^T-chunk   -> PSUM bank
  Mode index lands on partitions, so bias is a per-partition scalar and no
  stride-2 de-interleave is needed.  Tail per chunk pair:
      ACT   : sqx = Square(mux^T)              (PSUM f32 -> SBUF bf16)
      DVE   : sqp = mup^T * mup^T              (PSUM f32 -> SBUF bf16)
      DVE/GPSIMD (alternating): out = (sqx + bias) + sqp   (one
              scalar_tensor_tensor, all-bf16 SBUF -> DVE 4x mode)
  IO is fp16 in / bf16 out, halving HBM traffic vs f32 (DMA floor ~23us/core).
  Host-simulated pipeline rel err vs f64 reference: 8.0e-3 (gate 2e-2).
  Input pieces ride the SP HWDGE queue, output pieces the ACT HWDGE queue,
  4KB contiguous per partition per piece.
"""

import ml_dtypes
import numpy as np

import concourse.bass as bass
import concourse.mybir as mybir
import concourse.tile as tile
from concourse import bacc
from concourse.bass_utils import run_bass_kernel_spmd

N_QUMODES = 128
N_LAYERS = 8
BATCH = 131072
N_CORES = 8
ROWS = BATCH // N_CORES          # 16384 batch columns per core (free dim)
PIECES = 8                       # DMA granularity: 2048 cols = 4KB/partition
PC = ROWS // PIECES              # 2048
CHUNK = 512                      # matmul free dim (one PSUM bank of f32)
PAIRS = ROWS // (2 * CHUNK)      # 16 chunk pairs
F32 = mybir.dt.float32
F16 = mybir.dt.float16
BF16 = mybir.dt.bfloat16


def host_prep(params: np.ndarray):
    """Build Mxp [128, 256] fp16 (Mx | Mp) and bias [128, 1] f32 on host."""
    L, N = N_LAYERS, N_QUMODES
    p = params.reshape(L, N, 3).astype(np.float64)
    th1, r, th2 = p[..., 0], p[..., 1], p[..., 2]

    def rot(th):
        c, s = np.cos(th), np.sin(th)
        return np.stack([np.stack([c, -s], -1), np.stack([s, c], -1)], -2)

    z = np.zeros_like(r)
    sq = np.stack([np.stack([np.exp(-r), z], -1),
                   np.stack([z, np.exp(r)], -1)], -2)
    blk = np.einsum('lnab,lnbc,lncd->lnad', rot(th2), sq, rot(th1))

    t = np.cos(np.pi / 4)
    rr = np.sin(np.pi / 4)
    BS4 = np.array([[t, 0., -rr, 0.],
                    [0., t, 0., -rr],
                    [rr, 0., t, 0.],
                    [0., rr, 0., t]])
    C = np.eye(2 * N)
    for i in range(N - 1):
        C[2 * i:2 * i + 4, :] = BS4 @ C[2 * i:2 * i + 4, :]

    S = np.eye(2 * N)
    idx = np.arange(N)
    for l in range(L):
        D = np.zeros((N, 2, N, 2))
        D[idx, :, idx, :] = blk[l]
        S = C @ (D.reshape(2 * N, 2 * N) @ S)

    # mu_dev[b, j] = (inputs @ Ms)[b, j] with Ms = S[:, 0::2].T  [128, 256].
    Ms = S[:, 0::2].T
    Mx = Ms[:, 0::2]                 # [128 feat, 128 mode] x-quadrature
    Mp = Ms[:, 1::2]                 # p-quadrature
    mxp = np.ascontiguousarray(
        np.concatenate([Mx, Mp], axis=1)).astype(np.float16)   # [128, 256]

    dV = (S ** 2).sum(axis=1)                                  # [256]
    bias = ((dV[0::2] + dV[1::2]) / 4.0 - 0.5)
    bias_col = np.ascontiguousarray(bias.reshape(128, 1)).astype(np.float32)
    return mxp, bias_col


def build_bass():
    nc = bacc.Bacc("TRN2", target_bir_lowering=False, debug=False,
                   num_devices=N_CORES)

    x_d = nc.dram_tensor("x", [128, ROWS], F16, kind="ExternalInput")
    mxp_d = nc.dram_tensor("mxp", [128, 256], F16, kind="ExternalInput")
    bias_d = nc.dram_tensor("bias", [128, 1], F32, kind="ExternalInput")
    out_d = nc.dram_tensor("out", [128, ROWS], BF16, kind="ExternalOutput")

    x_v = x_d.ap().rearrange("p (k c) -> k p c", c=PC)     # [8][128, 2048]
    out_v = out_d.ap().rearrange("p (k c) -> k p c", c=PC)

    with tile.TileContext(nc) as tc:
        with (
            tc.tile_pool(name="const", bufs=1) as const_pool,
            tc.tile_pool(name="xin", bufs=PIECES) as xin_pool,
            tc.tile_pool(name="oout", bufs=4) as oout_pool,
            tc.tile_pool(name="sq", bufs=4) as sq_pool,
            tc.tile_pool(name="mup", bufs=2, space="PSUM") as mup_pool,
        ):
            mxp_sb = const_pool.tile([128, 256], F16)
            nc.sync.dma_start(out=mxp_sb, in_=mxp_d.ap())
            bias_sb = const_pool.tile([128, 1], F32)
            nc.sync.dma_start(out=bias_sb, in_=bias_d.ap())

            x_tiles = []
            for k in range(PIECES):
                x_sb = xin_pool.tile([128, PC], F16, tag="x_sb",
                                     name=f"x_sb_{k}")
                if k == 0:
                    # halve the first transfer so the PE can start sooner
                    nc.sync.dma_start(out=x_sb[:, 0:PC // 2],
                                      in_=x_v[k][:, 0:PC // 2])
                    nc.sync.dma_start(out=x_sb[:, PC // 2:],
                                      in_=x_v[k][:, PC // 2:])
                else:
                    nc.sync.dma_start(out=x_sb, in_=x_v[k])
                x_tiles.append(x_sb)

            out_tiles = {}
            for g in range(PAIRS):
                k, gk = divmod(g, PAIRS // PIECES)    # piece idx, pair in piece
                if gk == 0:
                    out_tiles[k] = oout_pool.tile([128, 4, CHUNK], BF16,
                                                  tag="o_sb", name=f"o_sb_{k}")
                x_sb = x_tiles[k]
                c0 = 2 * gk * CHUNK                   # col offset in piece
                c1 = c0 + CHUNK

                # 4 matmuls per pair (512 f32 = one PSUM bank is the ISA
                # max): banks [mux0, mux1, mup0, mup1], same-weight matmuls
                # adjacent to minimize PE weight reloads.
                ps = mup_pool.tile([128, 4, CHUNK], F32)
                nc.tensor.matmul(ps[:, 0, :], mxp_sb[:, 0:128],
                                 x_sb[:, c0:c0 + CHUNK], start=True, stop=True)
                nc.tensor.matmul(ps[:, 1, :], mxp_sb[:, 0:128],
                                 x_sb[:, c1:c1 + CHUNK], start=True, stop=True)
                nc.tensor.matmul(ps[:, 2, :], mxp_sb[:, 128:256],
                                 x_sb[:, c0:c0 + CHUNK], start=True, stop=True)
                nc.tensor.matmul(ps[:, 3, :], mxp_sb[:, 128:256],
                                 x_sb[:, c1:c1 + CHUNK], start=True, stop=True)

                # ONE ACT pass squares all 4 banks (single PSUM input AP),
                # then ONE DVE fused combine per pair.  Minimizing the
                # instruction count keeps the engine queues off the
                # semaphore-processing floor and lets the power governor
                # relax (fewer concurrently-hot engines).
                sq = sq_pool.tile([128, 4, CHUNK], BF16, tag="sq",
                                  name=f"sq_{g}")
                nc.scalar.activation(sq, ps,
                                     mybir.ActivationFunctionType.Square)
                o_sb = out_tiles[k]
                nc.vector.scalar_tensor_tensor(
                    out=o_sb[:, 2 * gk:2 * gk + 2, :], in0=sq[:, 0:2, :],
                    scalar=bias_sb, in1=sq[:, 2:4, :],
                    op0=mybir.AluOpType.add, op1=mybir.AluOpType.add)

                if gk == PAIRS // PIECES - 1:
                    # Outputs ride the SP queue too: in+out serialize there
                    # at exactly the aggregate DMA floor, and the ACT
                    # sequencer keeps all its time for the squares.
                    nc.sync.dma_start(out=out_v[k],
                                      in_=out_tiles.pop(k).rearrange(
                                          "p a b -> p (a b)"))

    nc.compile()
    return nc


_NC_CACHE = None


def make_in_maps(X: np.ndarray, params: np.ndarray):
    mxp, bias_col = host_prep(params)
    xt = np.ascontiguousarray(X.astype(np.float16).T)     # [128, BATCH]
    return [
        {"x": np.ascontiguousarray(xt[:, i * ROWS:(i + 1) * ROWS]),
         "mxp": mxp, "bias": bias_col}
        for i in range(N_CORES)
    ]


def assemble_output(results) -> np.ndarray:
    full = np.concatenate([r["out"] for r in results], axis=1)  # [128, BATCH]
    return np.ascontiguousarray(full.T.astype(np.float32))


def kernel(**inputs: np.ndarray) -> np.ndarray:
    global _NC_CACHE
    X = np.asarray(inputs["inputs"], dtype=np.float32)
    params = np.asarray(inputs["params"], dtype=np.float32)
    assert X.shape == (BATCH, N_QUMODES)

    if _NC_CACHE is None:
        _NC_CACHE = build_bass()
    nc = _NC_CACHE

    in_maps = make_in_maps(X, params)
    res = run_bass_kernel_spmd(nc, in_maps, core_ids=list(range(N_CORES)))
    return assemble_output(res.results)
